# revision 5
# baseline (speedup 1.0000x reference)
"""Trainium2 Bass kernel for nn_DIE: per-pixel channel SE gate.

    h    = relu(W1 @ x[:, :, i, j])      # [B, 32, H, W]
    gate = sigmoid(W2 @ h)               # [B, 512, H, W]
    y    = gate * x

Sharding: pure data parallel over the batch dim (B=8 -> 8 cores).
Each core streams its [512, 192*192] slab through SBUF in DMA tiles
of 1024 pixels (4KB contiguous runs per channel). Matmuls run in
float32r mode (fp32 bits, single-pass PE streaming at 1 row/cycle
for moving dim >= 256, ~4x faster than plain fp32 matmul; ~1.9e-4
absmax rounding error in the gate). The final elementwise multiply
uses the fp32 x exactly.
"""

import sys

for _p in ("/opt/trn_rl_repo",):
    if _p not in sys.path:
        sys.path.insert(0, _p)

import numpy as np

import concourse.bacc as bacc
import concourse.bass as bass
import concourse.mybir as mybir
from concourse import tile
from concourse.bass_utils import run_bass_kernel_spmd

B, C, H, W = 8, 512, 192, 192
R = 32            # C // RED
NPIX = H * W      # 36864 pixels per batch element
N_CORES = 8
DMA_N = 2048      # pixels per DMA tile
SUB_N = 512       # pixels per compute sub-tile (one PSUM bank of fp32)
PART = 128
G = C // PART     # 4 channel groups

F32 = mybir.dt.float32
F32R = mybir.dt.float32r
AF = mybir.ActivationFunctionType


def build(npix: int = NPIX, dma_n: int = DMA_N):
    """Build the per-core Bass program (SPMD: identical on all cores)."""
    assert npix % dma_n == 0 and dma_n % SUB_N == 0
    n_tiles = npix // dma_n
    n_sub = dma_n // SUB_N

    nc = bacc.Bacc("TRN2", target_bir_lowering=False, debug=False, num_devices=N_CORES)

    # float32r carries plain fp32 bits; declaring the DRAM side f32r makes
    # the DMA the "rounding producer" the BIR verifier requires for f32r
    # matmul operands.
    x_d = nc.dram_tensor("x", [C, npix], F32R, kind="ExternalInput").ap()
    w1t_d = nc.dram_tensor("w1t", [C, R], F32R, kind="ExternalInput").ap()  # W1.T
    w2t_d = nc.dram_tensor("w2t", [R, C], F32R, kind="ExternalInput").ap()  # W2.T
    y_d = nc.dram_tensor("y", [C, npix], F32, kind="ExternalOutput").ap()

    with tile.TileContext(nc) as tc:
        with (
            tc.tile_pool(name="wpool", bufs=1) as wpool,
            tc.tile_pool(name="xp", bufs=2) as xp,
            tc.tile_pool(name="hp", bufs=4) as hp,
            tc.tile_pool(name="gp", bufs=8) as gp,
            tc.tile_pool(name="op", bufs=2) as op_,
            tc.tile_pool(name="hpsum", bufs=2, space=bass.MemorySpace.PSUM) as hpsum,
            tc.tile_pool(name="gpsum", bufs=4, space=bass.MemorySpace.PSUM) as gpsum,
        ):
            # Weights, loaded once.
            # w1t[p, g, r] = W1T[g*128+p, r]; w2t[p, g, m] = W2T[p, g*128+m].
            w1t = wpool.tile([PART, G, R], F32R)
            nc.sync.dma_start(w1t[:], w1t_d.rearrange("(g p) r -> p g r", p=PART))
            w2t = wpool.tile([R, G, PART], F32R)
            nc.sync.dma_start(w2t[:], w2t_d.rearrange("r (g m) -> r g m", m=PART))

            for t in range(n_tiles):
                n0 = t * dma_n
                xt = xp.tile([PART, G, dma_n], F32R, tag="xt")
                nc.sync.dma_start(
                    xt[:],
                    x_d[:, n0 : n0 + dma_n].rearrange("(g p) n -> p g n", p=PART),
                )

                ot = op_.tile([PART, G, dma_n], F32, tag="ot")
                for s in range(n_sub):
                    sl = slice(s * SUB_N, (s + 1) * SUB_N)
                    # h[r, n] = sum_c W1[r, c] x[c, n], accumulated over chunks
                    hps = hpsum.tile([R, SUB_N], F32, tag="hps")
                    for g in range(G):
                        nc.tensor.matmul(
                            hps[:], w1t[:, g, :], xt[:, g, sl],
                            start=(g == 0), stop=(g == G - 1),
                        )
                    hs = hp.tile([R, SUB_N], F32R, tag="hs")
                    nc.scalar.activation(hs[:], hps[:], AF.Relu)

                    for g in range(G):
                        gps = gpsum.tile([PART, SUB_N], F32, tag="gps")
                        nc.tensor.matmul(
                            gps[:], w2t[:, g, :], hs[:], start=True, stop=True
                        )
                        gs = gp.tile([PART, SUB_N], F32, tag="gs")
                        nc.scalar.activation(gs[:], gps[:], AF.Sigmoid)
                        nc.vector.tensor_mul(
                            ot[:, g, sl], gs[:], xt[:, g, sl].bitcast(F32)
                        )

                nc.scalar.dma_start(
                    y_d[:, n0 : n0 + dma_n].rearrange("(g p) n -> p g n", p=PART),
                    ot[:],
                )

    nc.compile()
    return nc


def kernel(x: np.ndarray, W1: np.ndarray, W2: np.ndarray, **run_kwargs):
    """Full-input entry point: shards batch over 8 cores, returns full output."""
    assert x.shape == (B, C, H, W), x.shape
    nc = build()

    w1t = np.ascontiguousarray(W1.T.astype(np.float32))  # [512, 32]
    w2t = np.ascontiguousarray(W2.T.astype(np.float32))  # [32, 512]
    in_maps = [
        {
            "x": np.ascontiguousarray(x[i].reshape(C, NPIX).astype(np.float32)),
            "w1t": w1t,
            "w2t": w2t,
        }
        for i in range(N_CORES)
    ]
    res = run_bass_kernel_spmd(nc, in_maps, list(range(N_CORES)), **run_kwargs)
    y = np.stack([res.results[i]["y"].reshape(C, H, W) for i in range(N_CORES)])
    if run_kwargs:
        return y, res
    return y


# revision 6
# speedup vs baseline: 1.5176x; 1.5176x over previous
"""Trainium2 Bass kernel for nn_DIE: per-pixel channel SE gate.

    h    = relu(W1 @ x[:, :, i, j])      # [B, 32, H, W]
    gate = sigmoid(W2 @ h)               # [B, 512, H, W]
    y    = gate * x

Sharding: pure data parallel over the batch dim (B=8 -> 8 cores).
Each core streams its [512, 192*192] slab through SBUF in DMA tiles
of 1024 pixels (4KB contiguous runs per channel). Matmuls run in
float32r mode (fp32 bits, single-pass PE streaming at 1 row/cycle
for moving dim >= 256, ~4x faster than plain fp32 matmul; ~1.9e-4
absmax rounding error in the gate). The final elementwise multiply
uses the fp32 x exactly.
"""

import sys

for _p in ("/opt/trn_rl_repo",):
    if _p not in sys.path:
        sys.path.insert(0, _p)

import numpy as np

import concourse.bacc as bacc
import concourse.bass as bass
import concourse.mybir as mybir
from concourse import tile
from concourse.bass_utils import run_bass_kernel_spmd

B, C, H, W = 8, 512, 192, 192
R = 32            # C // RED
NPIX = H * W      # 36864 pixels per batch element
N_CORES = 8
DMA_N = 1024      # pixels per DMA tile
SUB_N = 512       # pixels per compute sub-tile (one PSUM bank of fp32)
PART = 128
G = C // PART     # 4 channel groups

F32 = mybir.dt.float32
F32R = mybir.dt.float32r
AF = mybir.ActivationFunctionType


def build(npix: int = NPIX, dma_n: int = DMA_N):
    """Build the per-core Bass program (SPMD: identical on all cores)."""
    assert npix % dma_n == 0 and dma_n % SUB_N == 0
    n_tiles = npix // dma_n
    n_sub = dma_n // SUB_N

    nc = bacc.Bacc("TRN2", target_bir_lowering=False, debug=False, num_devices=N_CORES)

    # float32r carries plain fp32 bits; declaring the DRAM side f32r makes
    # the DMA the "rounding producer" the BIR verifier requires for f32r
    # matmul operands.
    x_d = nc.dram_tensor("x", [C, npix], F32R, kind="ExternalInput").ap()
    w1t_d = nc.dram_tensor("w1t", [C, R], F32R, kind="ExternalInput").ap()  # W1.T
    w2t_d = nc.dram_tensor("w2t", [R, C], F32R, kind="ExternalInput").ap()  # W2.T
    y_d = nc.dram_tensor("y", [C, npix], F32, kind="ExternalOutput").ap()

    with tile.TileContext(nc) as tc:
        with (
            tc.tile_pool(name="wpool", bufs=1) as wpool,
            tc.tile_pool(name="xp", bufs=4) as xp,
            tc.tile_pool(name="hp", bufs=4) as hp,
            tc.tile_pool(name="gp", bufs=8) as gp,
            tc.tile_pool(name="op", bufs=4) as op_,
            tc.tile_pool(name="hpsum", bufs=2, space=bass.MemorySpace.PSUM) as hpsum,
            tc.tile_pool(name="gpsum", bufs=4, space=bass.MemorySpace.PSUM) as gpsum,
        ):
            # Weights, loaded once.
            # w1t[p, g, r] = W1T[g*128+p, r]; w2t[p, g, m] = W2T[p, g*128+m].
            w1t = wpool.tile([PART, G, R], F32R)
            nc.sync.dma_start(w1t[:], w1t_d.rearrange("(g p) r -> p g r", p=PART))
            w2t = wpool.tile([R, G, PART], F32R)
            nc.sync.dma_start(w2t[:], w2t_d.rearrange("r (g m) -> r g m", m=PART))

            for t in range(n_tiles):
                n0 = t * dma_n
                xt = xp.tile([PART, G, dma_n], F32R, tag="xt")
                nc.sync.dma_start(
                    xt[:],
                    x_d[:, n0 : n0 + dma_n].rearrange("(g p) n -> p g n", p=PART),
                )

                ot = op_.tile([PART, G, dma_n], F32, tag="ot")
                for s in range(n_sub):
                    sl = slice(s * SUB_N, (s + 1) * SUB_N)
                    # h[r, n] = sum_c W1[r, c] x[c, n], accumulated over chunks
                    hps = hpsum.tile([R, SUB_N], F32, tag="hps")
                    for g in range(G):
                        nc.tensor.matmul(
                            hps[:], w1t[:, g, :], xt[:, g, sl],
                            start=(g == 0), stop=(g == G - 1),
                        )
                    hs = hp.tile([R, SUB_N], F32R, tag="hs")
                    nc.scalar.activation(hs[:], hps[:], AF.Relu)

                    for g in range(G):
                        gps = gpsum.tile([PART, SUB_N], F32, tag="gps")
                        nc.tensor.matmul(
                            gps[:], w2t[:, g, :], hs[:], start=True, stop=True
                        )
                        gs = gp.tile([PART, SUB_N], F32, tag="gs")
                        nc.scalar.activation(gs[:], gps[:], AF.Sigmoid)
                        nc.vector.tensor_mul(
                            ot[:, g, sl], gs[:], xt[:, g, sl].bitcast(F32)
                        )

                nc.scalar.dma_start(
                    y_d[:, n0 : n0 + dma_n].rearrange("(g p) n -> p g n", p=PART),
                    ot[:],
                )

    nc.compile()
    return nc


def kernel(x: np.ndarray, W1: np.ndarray, W2: np.ndarray, **run_kwargs):
    """Full-input entry point: shards batch over 8 cores, returns full output."""
    assert x.shape == (B, C, H, W), x.shape
    nc = build()

    w1t = np.ascontiguousarray(W1.T.astype(np.float32))  # [512, 32]
    w2t = np.ascontiguousarray(W2.T.astype(np.float32))  # [32, 512]
    in_maps = [
        {
            "x": np.ascontiguousarray(x[i].reshape(C, NPIX).astype(np.float32)),
            "w1t": w1t,
            "w2t": w2t,
        }
        for i in range(N_CORES)
    ]
    res = run_bass_kernel_spmd(nc, in_maps, list(range(N_CORES)), **run_kwargs)
    y = np.stack([res.results[i]["y"].reshape(C, H, W) for i in range(N_CORES)])
    if run_kwargs:
        return y, res
    return y
